# revision 11
# baseline (speedup 1.0000x reference)
"""Causal multi-head attention on 8 Trainium2 NeuronCores.

Problem: B=2, S=2048, D=1024, H=16 heads (HD=64), fp32 I/O.
Sharding: batch x head-group. Core c handles batch c//4 and heads
4*(c%4) .. 4*(c%4)+3 (a 256-wide feature slice of Wq/Wk/Wv columns and
Wo rows). Each core writes a partial output projection for its batch;
the host sums the 4 partials per batch and adds the bias.

Device dataflow is fully "feature-major" (transposed) so no transposes
are ever needed on device:
  - host feeds x[b].T as xT [D, S]
  - QT = Wq_g.T @ xT (via matmul(lhsT=Wq chunk, rhs=xT chunk))  [256, S]
  - KT likewise; V in natural token-major layout via lhsT=xT chunks,
    with a ones-column appended per head (V_aug [S, 65]) so the ctx
    matmul's row 64 accumulates the softmax denominator for free
  - scores^T chunks [128 keys, 512 queries] = matmul(lhsT=KT chunk,
    rhs=QT tile) with K=64 contraction; two heads of a pair run as
    row-packed matmuls at base partitions 0/64 (concurrent in the PE)
  - softmax without max-subtraction (inputs are unit-scale gaussians;
    exp cannot overflow): exp on ACT with scale=1/8 fused, causal mask
    applied as a 0/1 multiply only on diagonal-crossing chunks, fully
    masked chunks skipped entirely
  - ctx_aug^T [65, 512] accumulated over key chunks; row 64 = denom
  - normalize: reciprocal of denom row, broadcast across partitions via
    a ones-outer-product matmul, multiply on DVE
  - out^T partial [1024, S] = matmul(lhsT=Wo_g chunk, rhs=ctx^T)
Matmul inputs use dtype float32r (full fp32 storage, ~1.8e-4 matmul
rounding, 4x faster than strict fp32 on the PE).
"""

import numpy as np

B, S, D, H, HD = 2, 2048, 1024, 16, 64
NCORES = 8
GROUPS = 4               # head groups (cores per batch)
HPC = H // GROUPS        # heads per core = 4
DG = HPC * HD            # per-core feature width = 256
P = 128
QT = 512                 # query tile (free dim)
KC = 128                 # key chunk (partition dim)
NQT = S // QT            # 4 query tiles
NKC = S // KC            # 16 key chunks
KCH = D // P             # 8 contraction chunks for projections
MCH = DG // P            # 2 feature chunks per core (= head pairs)
OCH = D // P             # 8 output feature chunks

_compiled = None


def _build(nreps=1):
    import concourse.bass as bass
    import concourse.tile as tile
    from concourse import bacc, mybir

    f32 = mybir.dt.float32
    f32r = mybir.dt.float32r
    EXP = mybir.ActivationFunctionType.Exp

    nc = bacc.Bacc("TRN2", target_bir_lowering=False, debug=False,
                   num_devices=NCORES)

    xT_d = nc.dram_tensor("xT", [D, S], f32r, kind="ExternalInput").ap()
    wq_d = nc.dram_tensor("wq", [D, DG], f32r, kind="ExternalInput").ap()
    wk_d = nc.dram_tensor("wk", [D, DG], f32r, kind="ExternalInput").ap()
    wv_d = nc.dram_tensor("wv", [D, DG], f32r, kind="ExternalInput").ap()
    wo_d = nc.dram_tensor("wo", [DG, D], f32r, kind="ExternalInput").ap()
    g_d = nc.dram_tensor("g", [P, QT + 3 * KC], f32r, kind="ExternalInput").ap()
    ones_d = nc.dram_tensor("ones", [P, HD], f32r, kind="ExternalInput").ap()
    out_d = nc.dram_tensor("outT", [D, S], f32, kind="ExternalOutput").ap()

    with tile.TileContext(nc) as tc:
        with tc.tile_pool(name="const", bufs=1) as const, \
             tc.tile_pool(name="work", bufs=3) as work, \
             tc.tile_pool(name="work2", bufs=2) as work2, \
             tc.tile_pool(name="psA", bufs=2, space="PSUM") as psA, \
             tc.tile_pool(name="psS", bufs=2, space="PSUM") as psS, \
             tc.tile_pool(name="psC", bufs=4, space="PSUM") as psC:

            xT = const.tile([P, KCH, S], f32r, tag="xT")
            wq = const.tile([P, KCH, DG], f32r, tag="wq")
            wk = const.tile([P, KCH, DG], f32r, tag="wk")
            wv = const.tile([P, KCH, DG], f32r, tag="wv")
            wo = const.tile([P, MCH, D], f32r, tag="wo")
            g = const.tile([P, QT + 3 * KC], f32r, tag="g")
            qT = const.tile([P, MCH, S], f32r, tag="qT")
            kT = const.tile([P, MCH, S], f32r, tag="kT")
            v = const.tile([P, NKC, HPC, HD + 1], f32r, tag="v")
            ctx = const.tile([P, MCH, S], f32r, tag="ctx")
            ones = const.tile([P, HD], f32r, tag="ones")

            # ---- input DMAs ----
            for c in range(KCH):
                nc.sync.dma_start(xT[:, c, :], xT_d[c * P:(c + 1) * P, :])
            for w_sb, w_dr in ((wq, wq_d), (wk, wk_d), (wv, wv_d)):
                nc.sync.dma_start(
                    w_sb[:], w_dr.rearrange("(c p) n -> p c n", p=P))
            nc.sync.dma_start(wo[:], wo_d.rearrange("(c p) n -> p c n", p=P))
            nc.sync.dma_start(g[:], g_d[:])
            nc.sync.dma_start(ones[:], ones_d[:])
            nc.sync.dma_start(
                v[:, :, :, HD:HD + 1],
                ones_d.rearrange("p (a b c) -> p a b c", a=NKC, b=HPC))

            def proj_qk(w_sb, t_sb, m, t):
                ps = psA.tile([P, QT], f32, tag="mm", name="psq")
                for k in range(KCH):
                    nc.tensor.matmul(
                        ps[:],
                        lhsT=w_sb[:, k, m * P:(m + 1) * P],
                        rhs=xT[:, k, t * QT:(t + 1) * QT],
                        start=(k == 0), stop=(k == KCH - 1))
                nc.vector.tensor_copy(t_sb[:, m, t * QT:(t + 1) * QT], ps[:])

            def proj_v(t):
                ps = psA.tile([P, QT], f32, tag="mm", name="psv")
                for k in range(KCH):
                    nc.tensor.matmul(
                        ps[:, :DG],
                        lhsT=xT[:, k, t * P:(t + 1) * P],
                        rhs=wv[:, k, :],
                        start=(k == 0), stop=(k == KCH - 1))
                nc.vector.tensor_copy(
                    v[:, t, :, 0:HD],
                    ps[:, :DG].rearrange("p (h d) -> p h d", h=HPC))

            def attn_block(pr, qi):
                qs = slice(qi * QT, (qi + 1) * QT)
                nkc = (qi + 1) * (QT // KC)
                cps = [psC.tile([HD + 1, QT], f32, tag="ctx",
                                name=f"ctx_{pr}_{qi}_{i}")
                       for i in range(2)]
                for kc in range(nkc):
                    diag = kc >= qi * (QT // KC)
                    for hh in range(2):
                        off = HD * hh
                        sps = psS.tile([P, QT], f32, tag="s", name="sps")
                        nc.tensor.matmul(
                            sps[:],
                            lhsT=kT[off:off + HD, pr, kc * KC:(kc + 1) * KC],
                            rhs=qT[off:off + HD, pr, qs])
                        es = work.tile([P, QT], f32r, tag="e", name="es")
                        nc.scalar.activation(es[:], sps[:], EXP,
                                             scale=1.0 / np.sqrt(HD))
                        if diag:
                            crel = kc - qi * (QT // KC)
                            goff = (QT - KC) - KC * crel
                            nc.gpsimd.tensor_mul(
                                es[:], es[:], g[:, goff:goff + QT])
                        nc.tensor.matmul(
                            cps[hh][:],
                            lhsT=v[:, kc, 2 * pr + hh, :],
                            rhs=es[:],
                            start=(kc == 0), stop=(kc == nkc - 1))
                for hh in range(2):
                    rt = work2.tile([HD + 1, QT], f32r, tag="r", name="rt")
                    with nc.allow_low_precision(reason="f32r matmul rhs"):
                        nc.vector.reciprocal(rt[HD:HD + 1, :],
                                             cps[hh][HD:HD + 1, :])
                    rbp = psA.tile([P, QT], f32, tag="mm", name="rbp")
                    nc.tensor.matmul(rbp[:HD, :],
                                     lhsT=ones[HD:HD + 1, :],
                                     rhs=rt[HD:HD + 1, :])
                    rbs = work2.tile([HD, QT], f32r, tag="rb", name="rbs")
                    nc.vector.tensor_copy(rbs[:], rbp[0:HD, :])
                    off = HD * hh
                    nc.vector.tensor_mul(
                        ctx[off:off + HD, pr, qs],
                        cps[hh][0:HD, :], rbs[:])

            def outproj(t):
                for m in range(OCH):
                    ps = psA.tile([P, QT], f32, tag="mm", name="pso")
                    for c in range(MCH):
                        nc.tensor.matmul(
                            ps[:],
                            lhsT=wo[:, c, m * P:(m + 1) * P],
                            rhs=ctx[:, c, t * QT:(t + 1) * QT],
                            start=(c == 0), stop=(c == MCH - 1))
                    st = work2.tile([P, QT], f32, tag="o", name="st")
                    nc.vector.tensor_copy(st[:], ps[:])
                    nc.sync.dma_start(
                        out_d[m * P:(m + 1) * P, t * QT:(t + 1) * QT], st[:])

            def phases():
                # pipeline-interleaved emission: V chunks and Q/K tiles for
                # query tile t arrive just before attention consumes them,
                # and the output projection for tile t follows immediately,
                # so PE proj work overlaps ACT-bound attention.
                for t in range(NQT):
                    for dt_ in range(QT // KC):
                        proj_v(t * (QT // KC) + dt_)
                    for m in range(MCH):
                        proj_qk(wq, qT, m, t)
                        proj_qk(wk, kT, m, t)
                    for pr in range(MCH):
                        attn_block(pr, t)
                    outproj(t)

            for _ in range(nreps):
                phases()

    nc.compile()
    return nc


def _mask():
    # G[k, j] = 1.0 iff k <= j - (QT - KC); slice [*, goff:goff+QT] gives
    # the 0/1 causal mask for a key chunk at relative offset crel within
    # a query tile: keep iff k + KC*crel <= q.
    j = np.arange(QT + 3 * KC)[None, :]
    k = np.arange(P)[:, None]
    return (k <= j - (QT - KC)).astype(np.float32)


def _in_maps(x, Wq, Wk, Wv, Wo):
    G = _mask()
    maps = []
    for c in range(NCORES):
        b, gidx = divmod(c, GROUPS)
        sl = slice(gidx * DG, (gidx + 1) * DG)
        maps.append({
            "xT": np.ascontiguousarray(x[b].T),
            "wq": np.ascontiguousarray(Wq[:, sl]),
            "wk": np.ascontiguousarray(Wk[:, sl]),
            "wv": np.ascontiguousarray(Wv[:, sl]),
            "wo": np.ascontiguousarray(Wo[sl, :]),
            "g": G,
            "ones": np.ones((P, HD), dtype=np.float32),
        })
    return maps


def kernel(x, Wq, Wk, Wv, Wo, bo):
    global _compiled
    from concourse.bass_utils import run_bass_kernel_spmd

    x = np.asarray(x, dtype=np.float32)
    Wq = np.asarray(Wq, dtype=np.float32)
    Wk = np.asarray(Wk, dtype=np.float32)
    Wv = np.asarray(Wv, dtype=np.float32)
    Wo = np.asarray(Wo, dtype=np.float32)
    bo = np.asarray(bo, dtype=np.float32)

    if _compiled is None:
        _compiled = _build()
    nc = _compiled

    res = run_bass_kernel_spmd(nc, _in_maps(x, Wq, Wk, Wv, Wo),
                               list(range(NCORES)))
    out = np.zeros((B, S, D), dtype=np.float32)
    for c in range(NCORES):
        out[c // GROUPS] += res.results[c]["outT"].T
    out += bo
    return out
